# revision 49
# baseline (speedup 1.0000x reference)
"""Trainium2 Bass kernel for nn_BasicNCAModel (neural cellular automaton).

Strategy (pure data parallelism, batch 16 -> 2 images per core x 8 cores):

* State layout per core: [128 partitions = 2 images x 64 channels,
  130 x 130 reflect-padded grid] in SBUF fp16, ping-pong buffered.
* The two depthwise 3x3 convs are folded into the hidden matmul:
  h = relu(sum_tap E_tap @ x_shift(tap) + b) with E_tap[256, 64].
* fp8 DoubleRow: 4 of the 9 taps pairs run as e4m3 DoubleRow matmuls that
  contract TWO taps per instruction (2 fp8 weights/PE cell).  The moving
  operand comes from packed fp8 state copies x8[b-idx][130 rows, 128 cols]
  (H-stride 128, no column halo) so a group's 512 pixels are contiguous;
  the K-pair stride (delta = d_bidx*16640 + d_a*128) is 16B-aligned as
  DoubleRow requires.  The W0-carrying center tap stays fp16 for accuracy;
  per-hidden-row scales (folded into bias and w_final) keep the e4m3
  weights in range.
* The mm2 matmuls (dx = wf.T @ h) for a finished group are deferred one
  group (emitted after the next group's mm1 stream) so the PE never
  waits on the relu chain; the two images' mm2 run concurrently on
  column tiles (0,0)/(0,64).  Slot order [single, 4x DR] per m-half
  measured fastest among five orderings (junction weight-load stalls
  are dominated by PE mode/tiling transitions, not load length).
* No per-step barrier: fp8-copy edge columns are batched per half-step,
  halo rows refresh right after the first/last group of each step, so
  consecutive steps pipeline on the PE without HAM re-throttle.
* Startup: a short burst of dummy matmuls on a memset tile warms the PE
  clock (HAM) while the initial state DMAs land; state chunks beyond the
  first are issued inside the first step's group loop so the t=0 gate
  tiles (fp8, 5-deep prefetch) don't starve behind them.
* Stochastic fire gate (pre-merged with the static life mask on the
  host, expanded to all 128 partitions, stored e4m3 0/1) streams in via
  DMA per group and is applied on DVE; step 0 applies the life mask to
  x0 via fp8 life tiles.
"""
import sys
sys.path.insert(0, '/opt/trn_rl_repo')

import numpy as np

B, H, W, C = 16, 128, 128, 64
HID = 256
STEPS = 8
NCORES = 8
BPC = B // NCORES            # images per core = 2
WP, HP = W + 2, H + 2        # padded grid 130 x 130
RPG = 4                      # W-rows per group
NPIX = RPG * H               # 512 pixels per matmul tile
NG = W // RPG                # 32 groups per step
CSZ = WP * H                 # 16640 elements per packed fp8 copy

NPAIRS = 4                   # fp8 DoubleRow tap pairs
NWARM = 18                   # PE warm-up dummy matmuls at kernel start

# tap schedule: pairs are e4m3 DoubleRow (2 taps/MM); center single fp16.
# b-copies: bidx b <-> cols b:b+128 of the padded state
NB = 3
BIDX = {0: 0, 1: 1, 2: 2}
PAIRS = [((0, 0), (1, 0)), ((2, 0), (0, 1)), ((2, 1), (0, 2)),
         ((1, 2), (2, 2))]
SINGLES = [(1, 1)]
NS = len(SINGLES)

_nc_cache = {}


def _build():
    import concourse.bacc as bacc
    import concourse.mybir as mybir
    import concourse.tile as tile
    from concourse.bass import AP

    F32 = mybir.dt.float32
    F16 = mybir.dt.float16
    BF16 = mybir.dt.bfloat16
    F8 = mybir.dt.float8e4
    AF = mybir.ActivationFunctionType
    ALU = mybir.AluOpType
    DR = mybir.MatmulPerfMode.DoubleRowSwInterleave

    nc = bacc.Bacc("TRN2", target_bir_lowering=False, debug=False,
                   enable_asserts=False, num_devices=NCORES)

    X0 = nc.dram_tensor("x0", [128, WP, HP], F16, kind="ExternalInput")
    X8 = nc.dram_tensor("x8", [128, NB, WP, H], F8, kind="ExternalInput")
    LG = nc.dram_tensor("lg", [NG, 128, NPIX], F16, kind="ExternalInput")
    WT8 = nc.dram_tensor("wt8", [128, NPAIRS, 2, 256], F8,
                         kind="ExternalInput")
    WTB = nc.dram_tensor("wtb", [128, NS, 2, 128], F16, kind="ExternalInput")
    WF = nc.dram_tensor("wf", [128, 2, 64], BF16, kind="ExternalInput")
    BI = nc.dram_tensor("bi", [128, 2], F32, kind="ExternalInput")
    GL = nc.dram_tensor("gl", [STEPS, NG, 128, NPIX], F8, kind="ExternalInput")
    OUT = nc.dram_tensor("out", [128, W, H], F16, kind="ExternalOutput")

    with tile.TileContext(nc) as tc:
        with tc.tile_pool(name="const", bufs=1) as cp, \
             tc.tile_pool(name="hbuf", bufs=3) as hp, \
             tc.tile_pool(name="gbuf", bufs=5) as gp, \
             tc.tile_pool(name="ph", bufs=1, space="PSUM") as php, \
             tc.tile_pool(name="pdx", bufs=2, space="PSUM") as pdxp:

            xA = cp.tile([128, WP, HP], F16, tag="xA")
            xB = cp.tile([128, WP, HP], F16, tag="xB")
            x8A = cp.tile([128, NB, WP, H], F8, tag="x8A")
            x8B = cp.tile([128, NB, WP, H], F8, tag="x8B")
            wt8 = cp.tile([128, NPAIRS, 2, 256], F8, tag="wt8")
            wtb = cp.tile([128, NS, 2, 128], F16, tag="wtb")
            wf = cp.tile([128, 2, 64], BF16, tag="wf")
            bi = cp.tile([128, 2], F32, tag="bi")
            warm = cp.tile([128, 512], BF16, tag="warm")

            nc.sync.dma_start(wt8[:], WT8[:])
            nc.sync.dma_start(wtb[:], WTB[:])
            nc.sync.dma_start(wf[:], WF[:])
            nc.sync.dma_start(bi[:], BI[:])

            # PE warm-up: HAM un-throttles after ~3.4us of sustained matmul
            # activity; burn that in on a zero tile while the state DMAs land
            nc.vector.memset(warm[:], 0.0)
            for _ in range(NWARM):
                wp = pdxp.tile([128, NPIX], F32, tag="dx")
                nc.tensor.matmul(wp[:], warm[:, 0:128], warm[:, 0:NPIX],
                                 start=True, stop=True, skip_group_check=True)

            # first state chunks up front; the rest are issued inside the
            # t=0 group loop so gate tiles don't starve behind them
            NCH = 10
            def chunk(c):
                r0, r1 = (WP * c) // NCH, (WP * (c + 1)) // NCH
                nc.sync.dma_start(xA[:, r0:r1, :], X0[:, r0:r1, :])
                nc.sync.dma_start(x8A[:, :, r0:r1, :], X8[:, :, r0:r1, :])
            chunk(0)

            def dr_rhs(x8s, img, w0, t1, t2):
                """[64, 2@delta, 512@1] moving AP for a DoubleRow tap pair."""
                (a1, b1), (a2, b2) = t1, t2
                delta = (BIDX[b2] - BIDX[b1]) * CSZ + (a2 - a1) * H
                assert delta > 0 and delta % 16 == 0, (t1, t2, delta)
                base = x8s[img * 64:(img + 1) * 64, BIDX[b1],
                           w0 + a1:w0 + a1 + RPG, :]
                ap = [list(base.ap[0]), [delta, 2], [1, NPIX]]
                return AP(base.tensor, base.offset, ap)

            def emit_mm2(p):
                """mm2 matmuls for a finished group; the returned dx psum is
                consumed by emit_rest one group later."""
                hA, hB, gate, life, xs, xd, x8d, w0, t = p
                dx = pdxp.tile([128, NPIX], F32, tag="dx")
                for k in range(2):
                    nc.tensor.matmul(dx[0:64, :], wf[:, k, :], hA[:, k, :],
                                     start=k == 0, stop=k == 1,
                                     skip_group_check=True)
                    nc.tensor.matmul(dx[64:128, :], wf[:, k, :], hB[:, k, :],
                                     start=k == 0, stop=k == 1,
                                     skip_group_check=True,
                                     tile_position=(0, 64))
                return dx

            def emit_rest(p, dx):
                """gate + state update + fp8 copy refresh for a finished
                group (issued one group later: PE never stalls)."""
                hA, hB, gate, life, xs, xd, x8d, w0, t = p
                first, last = t == 0, t == STEPS - 1
                tg = hp.tile([128, NPIX], F16, tag="tg")
                nc.vector.tensor_tensor(tg[:], dx[:], gate[:], ALU.mult)
                tg3 = tg[:].rearrange("p (a b) -> p a b", a=RPG)
                rows = slice(w0 + 1, w0 + 1 + RPG)
                if first:
                    # x1 = x0*L + dx*GL: the life tile holds host-premasked
                    # x0*L rows directly (fp16), so no on-device multiply --
                    # step 0's DVE load matches the steady steps
                    src3 = life[:].rearrange("p (a b) -> p a b", a=RPG)
                else:
                    src3 = xs[:, rows, 1:1 + H]
                nc.vector.tensor_tensor(xd[:, rows, 1:1 + H], tg3, src3,
                                        ALU.add)

                if last:
                    nc.sync.dma_start(OUT[:, w0:w0 + RPG, :],
                                      xd[:, rows, 1:1 + H])
                    return  # final state: fp8 copies never read again

                # packed fp8 b-copies; reflect halo columns read directly
                # (interior cols 1..128 only: nothing reads state cols 0/129).
                # Edge columns (b=0 pos 0 / b=2 pos 127) are batched per
                # half-step in the main loop; halo rows get theirs inline.
                def casts(dst_w, src_w, tiny):
                    nc.scalar.copy(x8d[:, 0, dst_w, 1:H], xd[:, src_w, 1:H])
                    if tiny:
                        nc.scalar.copy(x8d[:, 0, dst_w, 0:1], xd[:, src_w, 2:3])
                    nc.vector.tensor_copy(x8d[:, 1, dst_w, :],
                                          xd[:, src_w, 1:1 + H])
                    nc.vector.tensor_copy(x8d[:, 2, dst_w, 0:H - 1],
                                          xd[:, src_w, 2:1 + H])
                    if tiny:
                        nc.vector.tensor_copy(x8d[:, 2, dst_w, H - 1:H],
                                              xd[:, src_w, H - 1:H])

                casts(rows, rows, False)
                if w0 == 0:
                    # reflect row halos for next step's first group
                    nc.vector.tensor_copy(xd[:, 0, 1:1 + H],
                                          xd[:, 2, 1:1 + H])
                    casts(0, 2, True)
                if w0 == W - RPG:
                    nc.vector.tensor_copy(xd[:, WP - 1, 1:1 + H],
                                          xd[:, WP - 3, 1:1 + H])
                    casts(WP - 1, WP - 3, True)

            pends = []
            for t in range(STEPS):
                xs, xd = (xA, xB) if t % 2 == 0 else (xB, xA)
                x8s, x8d = (x8A, x8B) if t % 2 == 0 else (x8B, x8A)
                for g in range(NG):
                    w0 = RPG * g

                    # state chunks ahead of gate/life bytes in the DMA queue:
                    # chunks feed this step's matmuls, gates only next group's
                    # tail
                    if t == 0 and 1 <= g < NCH:
                        chunk(g)
                    gate = gp.tile([128, NPIX], F8, tag="gate")
                    nc.sync.dma_start(gate[:], GL[t, g])
                    life = None
                    if t == 0:
                        life = gp.tile([128, NPIX], F16, tag="life")
                        nc.sync.dma_start(life[:], LG[g])

                    # m0 PSUM double-buffered: the next group's first matmuls
                    # start without waiting on this group's m0 relu (8 banks:
                    # 2x2 m0 + 2x1 m1 + 2 dx)
                    phs = [[php.tile([128, NPIX], F32, tag=f"ph{im}{m}",
                                     name=f"ph{im}{m}", bufs=2 - m)
                            for m in range(2)] for im in range(2)]
                    hA = hp.tile([128, 2, NPIX], BF16, tag="hA")
                    hB = hp.tile([128, 2, NPIX], BF16, tag="hB")
                    # singles first: their tiny fp16 weight loads sit at the
                    # group junction so the DR loads prefetch behind them
                    # (measured: mid-chain, hoisted-pair, and m-seam-adjacent
                    # single orders all regress by 10-100us)
                    slots = ([("s", si) for si in range(NS)]
                             + [("p", pi) for pi in range(NPAIRS)])
                    for m in range(2):
                        nmm = len(slots)
                        for mi, (kind, idx) in enumerate(slots):
                            st, sp = mi == 0, mi == nmm - 1
                            for im in range(2):
                                if kind == "p":
                                    t1, t2 = PAIRS[idx]
                                    nc.tensor.matmul(
                                        phs[im][m][:],
                                        wt8[im * 64:(im + 1) * 64, idx, m],
                                        dr_rhs(x8s, im, w0, t1, t2),
                                        start=st, stop=sp, perf_mode=DR,
                                        skip_group_check=True)
                                else:
                                    a, b = SINGLES[idx]
                                    rhs = xs[im * 64:(im + 1) * 64,
                                             w0 + a:w0 + a + RPG, b:b + H]
                                    nc.tensor.matmul(
                                        phs[im][m][:],
                                        wtb[im * 64:(im + 1) * 64, idx, m],
                                        rhs, start=st, stop=sp,
                                        skip_group_check=True)
                        # relu + bias, PSUM -> SBUF bf16 (3 on ACT, 1 on DVE)
                        nc.scalar.activation(hA[:, m, :], phs[0][m][:], AF.Relu,
                                             bias=bi[:, m:m + 1])
                        if m == 0:
                            nc.scalar.activation(hB[:, m, :], phs[1][m][:],
                                                 AF.Relu, bias=bi[:, m:m + 1])
                        else:
                            nc.vector.tensor_scalar(
                                out=hB[:, m, :], in0=phs[1][m][:],
                                scalar1=bi[:, m:m + 1], scalar2=0.0,
                                op0=ALU.add, op1=ALU.max)

                    # one group's tail per group, deferred exactly one group
                    # (measured: batching two groups' mm2 back-to-back costs
                    # +80us -- every reordering of the PE instruction stream
                    # away from this schedule has regressed)
                    if pends:
                        p = pends.pop(0)
                        emit_rest(p, emit_mm2(p))
                    pends.append((hA, hB, gate, life, xs, xd, x8d, w0, t))

                    def tiny_batch(x8t, xt, r0, r1):
                        nc.scalar.copy(x8t[:, 0, r0:r1, 0:1], xt[:, r0:r1, 2:3])
                        nc.vector.tensor_copy(x8t[:, NB - 1, r0:r1, H - 1:H],
                                              xt[:, r0:r1, H - 1:H])

                    if g == 17 and t < STEPS - 1:
                        tiny_batch(x8d, xd, 1, NG * 2 + 1)
                    # bottom-half edge columns in four 16-row slices: one
                    # 64-row batch at g==1 overloads the FIFO ACT/DVE queues
                    # past the psum double-buffer slack (measured +1.3us bump
                    # at groups 1-2 every step); 16-row slices fit the
                    # per-group engine slack (earliest consumer: group 16)
                    if 1 <= t and g in (4, 7, 10, 13):
                        i = (4, 7, 10, 13).index(g)
                        r0 = NG * 2 + 1 + 16 * i
                        tiny_batch(x8s, xs, r0, r0 + 16)

            for p in pends:
                emit_rest(p, emit_mm2(p))

    nc.compile()
    return nc


def _host_pack(x, w_conv1, w_conv2, w_hidden, b_hidden, w_final, rand_vals):
    import ml_dtypes
    bf16 = ml_dtypes.bfloat16
    f16 = np.float16
    e4m3 = ml_dtypes.float8_e4m3

    Wh = np.asarray(w_hidden, np.float64)            # [256, 192]
    w1 = np.asarray(w_conv1, np.float64)[:, 0]       # [64, 3, 3]
    w2 = np.asarray(w_conv2, np.float64)[:, 0]

    E = {}
    for a in range(3):
        for b in range(3):
            Et = Wh[:, 64:128] * w1[None, :, a, b] + Wh[:, 128:192] * w2[None, :, a, b]
            if (a, b) == (1, 1):
                Et = Et + Wh[:, 0:64]
            E[(a, b)] = Et                            # [256, 64]

    fp8taps = [tp for pr in PAIRS for tp in pr]
    rowmax = np.max(np.stack([np.abs(E[tp]) for tp in fp8taps]), axis=(0, 2))
    s = np.clip(224.0 / np.maximum(rowmax, 1e-6), 0.25, 4096.0)

    # DoubleRowSwInterleave weight layout: per partition row the two K-tiles'
    # columns are pair-interleaved with M stored in reverse order:
    # flat[c, 2*(127-m) + i] = lhsT_i[c, m]
    wt8 = np.zeros((128, NPAIRS, 2, 256), np.float32)
    for pi, (t1, t2) in enumerate(PAIRS):
        for m in range(2):
            l0 = (E[t1] * s[:, None])[128 * m:128 * (m + 1), :].T     # [64,128]
            l1 = (E[t2] * s[:, None])[128 * m:128 * (m + 1), :].T
            flat = np.empty((64, 256), np.float32)
            flat[:, 0::2] = l0[:, ::-1]
            flat[:, 1::2] = l1[:, ::-1]
            wt8[0:64, pi, m, :] = flat
            wt8[64:128, pi, m, :] = flat
    wt8 = wt8.astype(e4m3)

    wtb = np.zeros((128, NS, 2, 128), np.float32)
    for si, tp in enumerate(SINGLES):
        Es = E[tp] * s[:, None]
        for m in range(2):
            lhsT = Es[128 * m:128 * (m + 1), :].T
            wtb[0:64, si, m, :] = lhsT
            wtb[64:128, si, m, :] = lhsT
    wtb = wtb.astype(f16)

    bv = np.asarray(b_hidden, np.float64) * s
    bi = np.stack([bv[0:128], bv[128:256]], axis=1).astype(np.float32)

    wfz = np.asarray(w_final, np.float64).copy()     # [64, 256]
    wfz[0:4, :] = 0.0                                # immutable image channels
    wfT = (wfz / s[None, :]).T                       # [256, 64]
    wf = np.ascontiguousarray(
        np.stack([wfT[0:128], wfT[128:256]], axis=1)).astype(bf16)

    # life mask is static: channel-0 updates masked out -> life == (x0 > 0)
    Lhw = np.asarray(x)[..., 0] > 0                  # [B, H, W]
    Lwh = np.ascontiguousarray(Lhw.transpose(0, 2, 1))   # [B, W, H]
    G = np.asarray(rand_vals)[..., 0] > 0.5          # [S, B, H, W]
    GLw = G.transpose(0, 1, 3, 2) & Lwh[None]        # [S, B, W, H]

    x_chw = np.asarray(x, np.float32).transpose(0, 3, 2, 1)      # [B, C, W, H]
    xp = np.pad(x_chw, ((0, 0), (0, 0), (1, 1), (1, 1)), mode='reflect')
    xp = xp.astype(f16)

    bvals = sorted(BIDX, key=lambda b: BIDX[b])
    in_maps = []
    for i in range(NCORES):
        sl = slice(BPC * i, BPC * (i + 1))
        x0 = np.ascontiguousarray(xp[sl].reshape(BPC * C, WP, HP))
        x8 = np.stack([x0[:, :, b:b + H] for b in bvals], axis=1)
        x8 = np.ascontiguousarray(x8).astype(e4m3)
        g2 = GLw[:, sl].reshape(STEPS, BPC, NG, NPIX).transpose(0, 2, 1, 3)
        glc = np.ascontiguousarray(
            np.broadcast_to(g2[:, :, :, None, :],
                            (STEPS, NG, BPC, 64, NPIX))
            .reshape(STEPS, NG, 128, NPIX)).astype(e4m3)
        # host-premasked x0*L inner rows per group: [NG, 128, 4*128] fp16
        x0m = (x_chw[sl] * Lwh[sl][:, None, :, :]).astype(np.float32)
        lgc = np.ascontiguousarray(
            x0m.reshape(BPC * C, NG, NPIX).transpose(1, 0, 2)).astype(f16)
        in_maps.append({
            "x0": x0, "x8": x8, "lg": lgc,
            "wt8": wt8, "wtb": wtb, "wf": wf, "bi": bi, "gl": glc,
        })
    return in_maps


def _run(inputs, trace=False, trace_kwargs=None):
    from concourse.bass_utils import run_bass_kernel_spmd
    if "nc" not in _nc_cache:
        _nc_cache["nc"] = _build()
    nc = _nc_cache["nc"]
    in_maps = _host_pack(
        inputs["x"], inputs["w_conv1"], inputs["w_conv2"], inputs["w_hidden"],
        inputs["b_hidden"], inputs["w_final"], inputs["rand_vals"])
    kwargs = {}
    if trace:
        kwargs["trace"] = True
        if trace_kwargs:
            kwargs.update(trace_kwargs)
    res = run_bass_kernel_spmd(nc, in_maps, core_ids=list(range(NCORES)), **kwargs)
    outs = []
    for i in range(NCORES):
        o = res.results[i]["out"].astype(np.float32).reshape(BPC, C, W, H)
        outs.append(o.transpose(0, 3, 2, 1))         # -> [b, H, W, C]
    full = np.concatenate(outs, axis=0).astype(np.float32)
    return full, res


def kernel(**inputs) -> np.ndarray:
    steps = int(np.asarray(inputs.get("steps", STEPS)))
    assert steps == STEPS, f"kernel compiled for {STEPS} steps, got {steps}"
    out, _ = _run(inputs)
    return out


# revision 51
# speedup vs baseline: 1.1912x; 1.1912x over previous
"""Trainium2 Bass kernel for nn_BasicNCAModel (neural cellular automaton).

Strategy (pure data parallelism, batch 16 -> 2 images per core x 8 cores):

* State layout per core: [128 partitions = 2 images x 64 channels,
  130 x 130 reflect-padded grid] in SBUF fp16, ping-pong buffered.
* The two depthwise 3x3 convs are folded into the hidden matmul:
  h = relu(sum_tap E_tap @ x_shift(tap) + b) with E_tap[256, 64].
* fp8 DoubleRow: 4 of the 9 taps pairs run as e4m3 DoubleRow matmuls that
  contract TWO taps per instruction (2 fp8 weights/PE cell).  The moving
  operand comes from packed fp8 state copies x8[b-idx][130 rows, 128 cols]
  (H-stride 128, no column halo) so a group's 512 pixels are contiguous;
  the K-pair stride (delta = d_bidx*16640 + d_a*128) is 16B-aligned as
  DoubleRow requires.  The W0-carrying center tap stays fp16 for accuracy;
  per-hidden-row scales (folded into bias and w_final) keep the e4m3
  weights in range.
* The mm2 matmuls (dx = wf.T @ h) for a finished group are deferred one
  group (emitted after the next group's mm1 stream) so the PE never
  waits on the relu chain; the two images' mm2 run concurrently on
  column tiles (0,0)/(0,64).  Slot order [single, 4x DR] per m-half
  measured fastest among five orderings (junction weight-load stalls
  are dominated by PE mode/tiling transitions, not load length).
* No per-step barrier: fp8-copy edge columns are batched per half-step,
  halo rows refresh right after the first/last group of each step, so
  consecutive steps pipeline on the PE without HAM re-throttle.
* Startup: a short burst of dummy matmuls on a memset tile warms the PE
  clock (HAM) while the initial state DMAs land; state chunks beyond the
  first are issued inside the first step's group loop so the t=0 gate
  tiles (fp8, 5-deep prefetch) don't starve behind them.
* Stochastic fire gate (pre-merged with the static life mask on the
  host, expanded to all 128 partitions, stored e4m3 0/1) streams in via
  DMA per group and is applied on DVE; step 0 applies the life mask to
  x0 via fp8 life tiles.
"""
import sys
sys.path.insert(0, '/opt/trn_rl_repo')

import numpy as np

B, H, W, C = 16, 128, 128, 64
HID = 256
STEPS = 8
NCORES = 8
BPC = B // NCORES            # images per core = 2
WP, HP = W + 2, H + 2        # padded grid 130 x 130
RPG = 4                      # W-rows per group
NPIX = RPG * H               # 512 pixels per matmul tile
NG = W // RPG                # 32 groups per step
CSZ = WP * H                 # 16640 elements per packed fp8 copy

NPAIRS = 4                   # fp8 DoubleRow tap pairs
NWARM = 18                   # PE warm-up dummy matmuls at kernel start

# tap schedule: pairs are e4m3 DoubleRow (2 taps/MM); center single fp16.
# b-copies: bidx b <-> cols b:b+128 of the padded state
NB = 3
BIDX = {0: 0, 1: 1, 2: 2}
PAIRS = [((0, 0), (1, 0)), ((2, 0), (0, 1)), ((2, 1), (0, 2)),
         ((1, 2), (2, 2))]
SINGLES = [(1, 1)]
NS = len(SINGLES)

_nc_cache = {}


def _build():
    import concourse.bacc as bacc
    import concourse.mybir as mybir
    import concourse.tile as tile
    from concourse.bass import AP

    F32 = mybir.dt.float32
    F16 = mybir.dt.float16
    BF16 = mybir.dt.bfloat16
    F8 = mybir.dt.float8e4
    AF = mybir.ActivationFunctionType
    ALU = mybir.AluOpType
    DR = mybir.MatmulPerfMode.DoubleRowSwInterleave

    nc = bacc.Bacc("TRN2", target_bir_lowering=False, debug=False,
                   enable_asserts=False, num_devices=NCORES)

    X0 = nc.dram_tensor("x0", [128, WP, HP], F16, kind="ExternalInput")
    X8 = nc.dram_tensor("x8", [128, NB, WP, H], F8, kind="ExternalInput")
    LG = nc.dram_tensor("lg", [NG, 128, NPIX], F16, kind="ExternalInput")
    WT8 = nc.dram_tensor("wt8", [128, NPAIRS, 2, 256], F8,
                         kind="ExternalInput")
    WTB = nc.dram_tensor("wtb", [128, NS, 2, 128], F16, kind="ExternalInput")
    WF = nc.dram_tensor("wf", [128, 2, 64], BF16, kind="ExternalInput")
    BI = nc.dram_tensor("bi", [128, 2], F32, kind="ExternalInput")
    GL = nc.dram_tensor("gl", [STEPS, NG, 128, NPIX], F8, kind="ExternalInput")
    OUT = nc.dram_tensor("out", [128, W, H], F16, kind="ExternalOutput")

    with tile.TileContext(nc) as tc:
        with tc.tile_pool(name="const", bufs=1) as cp, \
             tc.tile_pool(name="hbuf", bufs=3) as hp, \
             tc.tile_pool(name="gbuf", bufs=5) as gp, \
             tc.tile_pool(name="ph", bufs=1, space="PSUM") as php, \
             tc.tile_pool(name="pdx", bufs=2, space="PSUM") as pdxp:

            xA = cp.tile([128, WP, HP], F16, tag="xA")
            xB = cp.tile([128, WP, HP], F16, tag="xB")
            x8A = cp.tile([128, NB, WP, H], F8, tag="x8A")
            x8B = cp.tile([128, NB, WP, H], F8, tag="x8B")
            wt8 = cp.tile([128, NPAIRS, 2, 256], F8, tag="wt8")
            wtb = cp.tile([128, NS, 2, 128], F16, tag="wtb")
            wf = cp.tile([128, 2, 64], BF16, tag="wf")
            bi = cp.tile([128, 2], F32, tag="bi")
            warm = cp.tile([128, 512], BF16, tag="warm")

            nc.sync.dma_start(wt8[:], WT8[:])
            nc.sync.dma_start(wtb[:], WTB[:])
            nc.sync.dma_start(wf[:], WF[:])
            nc.sync.dma_start(bi[:], BI[:])

            # PE warm-up: HAM un-throttles after ~3.4us of sustained matmul
            # activity; burn that in on a zero tile while the state DMAs land
            nc.vector.memset(warm[:], 0.0)
            for _ in range(NWARM):
                wp = pdxp.tile([128, NPIX], F32, tag="dx")
                nc.tensor.matmul(wp[:], warm[:, 0:128], warm[:, 0:NPIX],
                                 start=True, stop=True, skip_group_check=True)

            # first state chunks up front; the rest are issued inside the
            # t=0 group loop so gate tiles don't starve behind them
            NCH = 10
            def chunk(c):
                r0, r1 = (WP * c) // NCH, (WP * (c + 1)) // NCH
                nc.sync.dma_start(xA[:, r0:r1, :], X0[:, r0:r1, :])
                nc.sync.dma_start(x8A[:, :, r0:r1, :], X8[:, :, r0:r1, :])
            chunk(0)

            def dr_rhs(x8s, img, w0, t1, t2):
                """[64, 2@delta, 512@1] moving AP for a DoubleRow tap pair."""
                (a1, b1), (a2, b2) = t1, t2
                delta = (BIDX[b2] - BIDX[b1]) * CSZ + (a2 - a1) * H
                assert delta > 0 and delta % 16 == 0, (t1, t2, delta)
                base = x8s[img * 64:(img + 1) * 64, BIDX[b1],
                           w0 + a1:w0 + a1 + RPG, :]
                ap = [list(base.ap[0]), [delta, 2], [1, NPIX]]
                return AP(base.tensor, base.offset, ap)

            def emit_mm2(p):
                """mm2 matmuls for a finished group; the returned dx psum is
                consumed by emit_rest one group later."""
                hA, hB, gate, life, xs, xd, x8d, w0, t = p
                dx = pdxp.tile([128, NPIX], F32, tag="dx")
                for k in range(2):
                    nc.tensor.matmul(dx[0:64, :], wf[:, k, :], hA[:, k, :],
                                     start=k == 0, stop=k == 1,
                                     skip_group_check=True)
                    nc.tensor.matmul(dx[64:128, :], wf[:, k, :], hB[:, k, :],
                                     start=k == 0, stop=k == 1,
                                     skip_group_check=True,
                                     tile_position=(0, 64))
                return dx

            def emit_rest(p, dx):
                """gate + state update + fp8 copy refresh for a finished
                group (issued one group later: PE never stalls)."""
                hA, hB, gate, life, xs, xd, x8d, w0, t = p
                first, last = t == 0, t == STEPS - 1
                tg = hp.tile([128, NPIX], F16, tag="tg")
                nc.vector.tensor_tensor(tg[:], dx[:], gate[:], ALU.mult)
                tg3 = tg[:].rearrange("p (a b) -> p a b", a=RPG)
                rows = slice(w0 + 1, w0 + 1 + RPG)
                if first:
                    # x1 = x0*L + dx*GL: the life tile holds host-premasked
                    # x0*L rows directly (fp16), so no on-device multiply --
                    # step 0's DVE load matches the steady steps
                    src3 = life[:].rearrange("p (a b) -> p a b", a=RPG)
                else:
                    src3 = xs[:, rows, 1:1 + H]
                nc.vector.tensor_tensor(xd[:, rows, 1:1 + H], tg3, src3,
                                        ALU.add)

                if last:
                    nc.sync.dma_start(OUT[:, w0:w0 + RPG, :],
                                      xd[:, rows, 1:1 + H])
                    return  # final state: fp8 copies never read again

                # packed fp8 b-copies; reflect halo columns read directly
                # (interior cols 1..128 only: nothing reads state cols 0/129).
                # Edge columns (b=0 pos 0 / b=2 pos 127) are batched per
                # half-step in the main loop; halo rows get theirs inline.
                def casts(dst_w, src_w, tiny):
                    nc.scalar.copy(x8d[:, 0, dst_w, 1:H], xd[:, src_w, 1:H])
                    if tiny:
                        nc.scalar.copy(x8d[:, 0, dst_w, 0:1], xd[:, src_w, 2:3])
                    nc.vector.tensor_copy(x8d[:, 1, dst_w, :],
                                          xd[:, src_w, 1:1 + H])
                    nc.vector.tensor_copy(x8d[:, 2, dst_w, 0:H - 1],
                                          xd[:, src_w, 2:1 + H])
                    if tiny:
                        nc.vector.tensor_copy(x8d[:, 2, dst_w, H - 1:H],
                                              xd[:, src_w, H - 1:H])

                casts(rows, rows, False)
                if w0 == 0:
                    # reflect row halos for next step's first group
                    nc.vector.tensor_copy(xd[:, 0, 1:1 + H],
                                          xd[:, 2, 1:1 + H])
                    casts(0, 2, True)
                if w0 == W - RPG:
                    nc.vector.tensor_copy(xd[:, WP - 1, 1:1 + H],
                                          xd[:, WP - 3, 1:1 + H])
                    casts(WP - 1, WP - 3, True)

            pends = []
            for t in range(STEPS):
                xs, xd = (xA, xB) if t % 2 == 0 else (xB, xA)
                x8s, x8d = (x8A, x8B) if t % 2 == 0 else (x8B, x8A)
                for g in range(NG):
                    w0 = RPG * g

                    # state chunks ahead of gate/life bytes in the DMA queue:
                    # chunks feed this step's matmuls, gates only next group's
                    # tail
                    if t == 0 and 1 <= g < NCH:
                        chunk(g)
                    gate = gp.tile([128, NPIX], F8, tag="gate")
                    nc.sync.dma_start(gate[:], GL[t, g])
                    life = None
                    if t == 0:
                        life = gp.tile([128, NPIX], F16, tag="life")
                        nc.sync.dma_start(life[:], LG[g])

                    # m0 PSUM double-buffered: the next group's first matmuls
                    # start without waiting on this group's m0 relu (8 banks:
                    # 2x2 m0 + 2x1 m1 + 2 dx)
                    phs = [[php.tile([128, NPIX], F32, tag=f"ph{im}{m}",
                                     name=f"ph{im}{m}", bufs=2 - m)
                            for m in range(2)] for im in range(2)]
                    hA = hp.tile([128, 2, NPIX], BF16, tag="hA")
                    hB = hp.tile([128, 2, NPIX], BF16, tag="hB")
                    # singles first: their tiny fp16 weight loads sit at the
                    # group junction so the DR loads prefetch behind them
                    # (measured: mid-chain, hoisted-pair, and m-seam-adjacent
                    # single orders all regress by 10-100us)
                    slots = ([("s", si) for si in range(NS)]
                             + [("p", pi) for pi in range(NPAIRS)])
                    for m in range(2):
                        nmm = len(slots)
                        for mi, (kind, idx) in enumerate(slots):
                            st, sp = mi == 0, mi == nmm - 1
                            for im in range(2):
                                if kind == "p":
                                    t1, t2 = PAIRS[idx]
                                    nc.tensor.matmul(
                                        phs[im][m][:],
                                        wt8[im * 64:(im + 1) * 64, idx, m],
                                        dr_rhs(x8s, im, w0, t1, t2),
                                        start=st, stop=sp, perf_mode=DR,
                                        skip_group_check=True)
                                else:
                                    a, b = SINGLES[idx]
                                    rhs = xs[im * 64:(im + 1) * 64,
                                             w0 + a:w0 + a + RPG, b:b + H]
                                    nc.tensor.matmul(
                                        phs[im][m][:],
                                        wtb[im * 64:(im + 1) * 64, idx, m],
                                        rhs, start=st, stop=sp,
                                        skip_group_check=True)
                        # relu + bias, PSUM -> SBUF bf16 (3 on ACT, 1 on DVE)
                        nc.scalar.activation(hA[:, m, :], phs[0][m][:], AF.Relu,
                                             bias=bi[:, m:m + 1])
                        if m == 0:
                            nc.scalar.activation(hB[:, m, :], phs[1][m][:],
                                                 AF.Relu, bias=bi[:, m:m + 1])
                        else:
                            nc.vector.tensor_scalar(
                                out=hB[:, m, :], in0=phs[1][m][:],
                                scalar1=bi[:, m:m + 1], scalar2=0.0,
                                op0=ALU.add, op1=ALU.max)

                    # one group's tail per group, deferred exactly one group
                    # (measured: batching two groups' mm2 back-to-back costs
                    # +80us -- every reordering of the PE instruction stream
                    # away from this schedule has regressed)
                    if pends:
                        p = pends.pop(0)
                        emit_rest(p, emit_mm2(p))
                    pends.append((hA, hB, gate, life, xs, xd, x8d, w0, t))

                    def tiny_batch(x8t, xt, r0, r1):
                        nc.scalar.copy(x8t[:, 0, r0:r1, 0:1], xt[:, r0:r1, 2:3])
                        nc.vector.tensor_copy(x8t[:, NB - 1, r0:r1, H - 1:H],
                                              xt[:, r0:r1, H - 1:H])

                    if g == 17 and t < STEPS - 1:
                        tiny_batch(x8d, xd, 1, NG * 2 + 1)
                    # at g==9 (not g==1): same two ops, but no longer stacked
                    # on top of tail(0)'s halo casts in the ACT/DVE FIFOs
                    # (earliest consumer of these edge columns is group 16)
                    if g == 9 and 1 <= t:
                        tiny_batch(x8s, xs, NG * 2 + 1, WP - 1)

            for p in pends:
                emit_rest(p, emit_mm2(p))

    nc.compile()
    return nc


def _host_pack(x, w_conv1, w_conv2, w_hidden, b_hidden, w_final, rand_vals):
    import ml_dtypes
    bf16 = ml_dtypes.bfloat16
    f16 = np.float16
    e4m3 = ml_dtypes.float8_e4m3

    Wh = np.asarray(w_hidden, np.float64)            # [256, 192]
    w1 = np.asarray(w_conv1, np.float64)[:, 0]       # [64, 3, 3]
    w2 = np.asarray(w_conv2, np.float64)[:, 0]

    E = {}
    for a in range(3):
        for b in range(3):
            Et = Wh[:, 64:128] * w1[None, :, a, b] + Wh[:, 128:192] * w2[None, :, a, b]
            if (a, b) == (1, 1):
                Et = Et + Wh[:, 0:64]
            E[(a, b)] = Et                            # [256, 64]

    fp8taps = [tp for pr in PAIRS for tp in pr]
    rowmax = np.max(np.stack([np.abs(E[tp]) for tp in fp8taps]), axis=(0, 2))
    s = np.clip(224.0 / np.maximum(rowmax, 1e-6), 0.25, 4096.0)

    # DoubleRowSwInterleave weight layout: per partition row the two K-tiles'
    # columns are pair-interleaved with M stored in reverse order:
    # flat[c, 2*(127-m) + i] = lhsT_i[c, m]
    wt8 = np.zeros((128, NPAIRS, 2, 256), np.float32)
    for pi, (t1, t2) in enumerate(PAIRS):
        for m in range(2):
            l0 = (E[t1] * s[:, None])[128 * m:128 * (m + 1), :].T     # [64,128]
            l1 = (E[t2] * s[:, None])[128 * m:128 * (m + 1), :].T
            flat = np.empty((64, 256), np.float32)
            flat[:, 0::2] = l0[:, ::-1]
            flat[:, 1::2] = l1[:, ::-1]
            wt8[0:64, pi, m, :] = flat
            wt8[64:128, pi, m, :] = flat
    wt8 = wt8.astype(e4m3)

    wtb = np.zeros((128, NS, 2, 128), np.float32)
    for si, tp in enumerate(SINGLES):
        Es = E[tp] * s[:, None]
        for m in range(2):
            lhsT = Es[128 * m:128 * (m + 1), :].T
            wtb[0:64, si, m, :] = lhsT
            wtb[64:128, si, m, :] = lhsT
    wtb = wtb.astype(f16)

    bv = np.asarray(b_hidden, np.float64) * s
    bi = np.stack([bv[0:128], bv[128:256]], axis=1).astype(np.float32)

    wfz = np.asarray(w_final, np.float64).copy()     # [64, 256]
    wfz[0:4, :] = 0.0                                # immutable image channels
    wfT = (wfz / s[None, :]).T                       # [256, 64]
    wf = np.ascontiguousarray(
        np.stack([wfT[0:128], wfT[128:256]], axis=1)).astype(bf16)

    # life mask is static: channel-0 updates masked out -> life == (x0 > 0)
    Lhw = np.asarray(x)[..., 0] > 0                  # [B, H, W]
    Lwh = np.ascontiguousarray(Lhw.transpose(0, 2, 1))   # [B, W, H]
    G = np.asarray(rand_vals)[..., 0] > 0.5          # [S, B, H, W]
    GLw = G.transpose(0, 1, 3, 2) & Lwh[None]        # [S, B, W, H]

    x_chw = np.asarray(x, np.float32).transpose(0, 3, 2, 1)      # [B, C, W, H]
    xp = np.pad(x_chw, ((0, 0), (0, 0), (1, 1), (1, 1)), mode='reflect')
    xp = xp.astype(f16)

    bvals = sorted(BIDX, key=lambda b: BIDX[b])
    in_maps = []
    for i in range(NCORES):
        sl = slice(BPC * i, BPC * (i + 1))
        x0 = np.ascontiguousarray(xp[sl].reshape(BPC * C, WP, HP))
        x8 = np.stack([x0[:, :, b:b + H] for b in bvals], axis=1)
        x8 = np.ascontiguousarray(x8).astype(e4m3)
        g2 = GLw[:, sl].reshape(STEPS, BPC, NG, NPIX).transpose(0, 2, 1, 3)
        glc = np.ascontiguousarray(
            np.broadcast_to(g2[:, :, :, None, :],
                            (STEPS, NG, BPC, 64, NPIX))
            .reshape(STEPS, NG, 128, NPIX)).astype(e4m3)
        # host-premasked x0*L inner rows per group: [NG, 128, 4*128] fp16
        x0m = (x_chw[sl] * Lwh[sl][:, None, :, :]).astype(np.float32)
        lgc = np.ascontiguousarray(
            x0m.reshape(BPC * C, NG, NPIX).transpose(1, 0, 2)).astype(f16)
        in_maps.append({
            "x0": x0, "x8": x8, "lg": lgc,
            "wt8": wt8, "wtb": wtb, "wf": wf, "bi": bi, "gl": glc,
        })
    return in_maps


def _run(inputs, trace=False, trace_kwargs=None):
    from concourse.bass_utils import run_bass_kernel_spmd
    if "nc" not in _nc_cache:
        _nc_cache["nc"] = _build()
    nc = _nc_cache["nc"]
    in_maps = _host_pack(
        inputs["x"], inputs["w_conv1"], inputs["w_conv2"], inputs["w_hidden"],
        inputs["b_hidden"], inputs["w_final"], inputs["rand_vals"])
    kwargs = {}
    if trace:
        kwargs["trace"] = True
        if trace_kwargs:
            kwargs.update(trace_kwargs)
    res = run_bass_kernel_spmd(nc, in_maps, core_ids=list(range(NCORES)), **kwargs)
    outs = []
    for i in range(NCORES):
        o = res.results[i]["out"].astype(np.float32).reshape(BPC, C, W, H)
        outs.append(o.transpose(0, 3, 2, 1))         # -> [b, H, W, C]
    full = np.concatenate(outs, axis=0).astype(np.float32)
    return full, res


def kernel(**inputs) -> np.ndarray:
    steps = int(np.asarray(inputs.get("steps", STEPS)))
    assert steps == STEPS, f"kernel compiled for {STEPS} steps, got {steps}"
    out, _ = _run(inputs)
    return out
